# revision 24
# baseline (speedup 1.0000x reference)
"""Two-layer GAT (single-head, PyG-style) + link predictor on 8 TRN2 NeuronCores.

Strategy (memory-regime):
  - Nodes sharded 8-way (6250/core, padded to 6272 = 49 windows of 128); edges
    assigned to the core owning their dst node so edge-softmax and the weighted
    scatter-sum are core-local.
  - Source features for non-self edges are fetched with batched dma_gather
    (InstDMAGatherAnt): 1024 rows per call, calls round-robined over 4 SWDGE
    queues. Gather indices are int16, so the halo table is split into two
    row-halves of 25024 rows; edge slots are partitioned by source half.
  - Segment softmax + weighted segment-sum run as one-hot matmuls on the PE:
        psum[dloc, :] += sum_slot p_slot * [dst_slot == dloc] * gt[slot, :]
    One-hot tiles are built with all-fp16 scalar_tensor_tensor ops.  The
    softmax denominators (and the link-predictor u/v dots in layer 2)
    accumulate via per-ref [128,K] matmuls into a single full-bank PSUM tile
    whose columns are per-window: the first matmul's start bit clears the
    whole bank, later windows accumulate into their own columns.  exp() needs
    no segment-max shift (logits are O(6), the shift cancels in the ratio).
  - Per-window feature sums are copied un-normalized to fp16; softmax
    normalize, bias and relu run as batched ops after the window loop.
  - Dense projections run sharded on PE in fp16 with extra folded columns
    [W | W@a_s | W@a_d (| W@wl0 | W@wl1)] emitted interleaved with h in one
    psum->SBUF copy per window.  The link predictor reduces to
    sigmoid(u[m0] + v[m1] + const) on host-gathered per-node scalars.
  - All host work between launches is index-space movement / layout shuffling
    (fp16 byte moves, int index prep); per-edge and per-node float math (exp,
    leaky-relu, softmax, dots) happens on device.

Launches: L1 proj1 -> L2 agg1 -> L3 proj2(+wl folds) -> L4 agg2(+u/v) -> L5 link.
"""
import time

import numpy as np

import concourse.bass as bass
import concourse.mybir as mybir
import concourse.tile as tile
from concourse import bacc
from concourse.bass_utils import run_bass_kernel_spmd

F32 = mybir.dt.float32
F16 = mybir.dt.float16
I16 = mybir.dt.int16

NCORES = 8
N, F_IN, H, C = 50000, 128, 256, 1
NS = N // NCORES            # 6250 nodes per shard
W = (NS + 127) // 128       # 49 windows per shard
NSP = W * 128               # 6272 padded slots
RH = 25024                  # rows per half table (int16-indexable)
NPAD = 2 * RH               # 50048 padded global rows
NEG = -1.0e30               # pad-edge sentinel (exp -> exactly 0)
CH = 8                      # tiles per dma_gather call (1024 rows = ring max)
NQ = 4                      # SWDGE queues
LOOKAHEAD = 3               # windows of gather prefetch
GBUFS = 8                   # gather ring buffers per region
PCH = 7                     # windows per projection load/store chunk

LAST_EXEC_NS = {}           # launch name -> exec_time_ns (filled per kernel() call)
_PROG_CACHE = {}


# ----------------------------------------------------------------- host prep
def _prep_graph(edge_index):
    """Partition non-self edges by dst shard, split by src half, sort by dst.
    Slots are tile-aligned per (window, half): window w's half-R edges occupy
    tiles [off_R(w), off_R(w)+nt_R[w]) of region R's tile space.  REF columns
    (dstf/esx/edx) are window-major: window w's columns are
    window-major (A tiles then B tiles per window)."""
    src = np.asarray(edge_index[0], np.int64)
    dst = np.asarray(edge_index[1], np.int64)
    core = dst // NS

    mA = np.zeros(W, np.int64)      # per-window max (over cores) edge counts
    mB = np.zeros(W, np.int64)
    per_core = []
    for c in range(NCORES):
        m = core == c
        s, dl = src[m], dst[m] - c * NS
        half = (s >= RH).astype(np.int64)
        w = dl // 128
        order = np.lexsort((dl, half * W + w))
        s, dl, half, w = s[order], dl[order], half[order], w[order]
        per_core.append((s, dl, half, w))
        for r, mm in ((0, mA), (1, mB)):
            cnt = np.bincount(w[half == r], minlength=W)
            mm[:] = np.maximum(mm, cnt)
    SA = np.concatenate([[0], np.cumsum(mA)]).astype(np.int64)
    SB = np.concatenate([[0], np.cumsum(mB)]).astype(np.int64)
    TA = int(-(-SA[-1] // 128))
    TB = int(-(-SB[-1] // 128))
    # REF columns, window-major: for w, tiles touched in A then in B
    refs = []                        # (w, region, tile)
    for w in range(W):
        if mA[w]:
            refs += [(w, 0, t) for t in range(SA[w] // 128,
                                              (SA[w] + mA[w] - 1) // 128 + 1)]
        if mB[w]:
            refs += [(w, 1, t) for t in range(SB[w] // 128,
                                              (SB[w] + mB[w] - 1) // 128 + 1)]
    NREF = len(refs)

    idxA = np.zeros((NCORES, 16, TA * 8), np.int16)
    idxB = np.zeros((NCORES, 16, TB * 8), np.int16)
    dstf = np.zeros((NCORES, 128, NREF), np.float16)
    esrc = np.zeros((NCORES, 128, NREF), np.int32)   # src node per slot (-1 pad)
    edst = np.zeros((NCORES, 128, NREF), np.int32)   # dst node (global) per slot
    esrc[:] = -1
    for c in range(NCORES):
        s, dl, half, w = per_core[c]
        for r, idx, S, TR in ((0, idxA, SA, TA), (1, idxB, SB, TB)):
            m = half == r
            sr, dlr, wr = s[m], dl[m], w[m]
            # rank within window
            slotsrc = np.full(TR * 128, -1, np.int64)
            slotdst = np.zeros(TR * 128, np.int64)
            first = np.searchsorted(wr, np.arange(W))
            rank = np.arange(len(wr)) - first[wr]
            slot = S[wr] + rank
            slotsrc[slot] = sr
            slotdst[slot] = dlr
            valid = slotsrc >= 0
            idx[c, np.nonzero(valid)[0] % 16, np.nonzero(valid)[0] // 16] = (
                (slotsrc[valid] - r * RH).astype(np.int16))
            for col, (wi, rr, t) in enumerate(refs):
                if rr != r:
                    continue
                sl = np.arange(t * 128, t * 128 + 128)
                inw = (sl >= S[wi]) & (sl < S[wi] + mA[wi] if rr == 0
                                       else (sl >= S[wi]) & (sl < S[wi] + mB[wi]))
                inw = (sl >= S[wi]) & (sl < S[wi] + (mA[wi] if rr == 0 else mB[wi]))
                ok = inw & (slotsrc[sl] >= 0)
                pp = np.nonzero(ok)[0]
                dstf[c, pp, col] = (slotdst[sl[pp]] - 128 * wi).astype(np.float16)
                esrc[c, pp, col] = slotsrc[sl[pp]]
                edst[c, pp, col] = slotdst[sl[pp]] + c * NS
    idxA = np.broadcast_to(idxA[:, None], (NCORES, 8, 16, TA * 8)).reshape(
        NCORES, 128, TA * 8)
    idxB = np.broadcast_to(idxB[:, None], (NCORES, 8, 16, TB * 8)).reshape(
        NCORES, 128, TB * 8)
    return dict(mA=tuple(int(x) for x in mA), mB=tuple(int(x) for x in mB),
                TA=TA, TB=TB, NT=NREF, refs=refs,
                idxA=np.ascontiguousarray(idxA),
                idxB=np.ascontiguousarray(idxB),
                dstf=dstf, esrc=esrc, edst=edst)


def _expand(es_full, ed_full, g, c):
    """Per-slot es[src], ed[dst] (+NEG sentinel for pads) and per-node
    self-loop es/ed in [128, W] layout.  Pure index-space gathers."""
    pad = g["esrc"][c] < 0
    esx = es_full[np.where(pad, 0, g["esrc"][c])].astype(np.float32)
    edx = ed_full[np.minimum(g["edst"][c], N - 1)].astype(np.float32)
    esx[pad] = NEG
    edx[pad] = 0.0
    nid = np.arange(NSP)
    nglob = np.minimum(c * NS + nid, N - 1)
    ess = np.where(nid < NS, es_full[nglob], 0.0).astype(np.float32)
    eds = np.where(nid < NS, ed_full[nglob], 0.0).astype(np.float32)
    return esx, edx, ess.reshape(W, 128).T.copy(), eds.reshape(W, 128).T.copy()


# ------------------------------------------------------------- bass programs
def _build_proj(kc, d_out, nfold):
    """Projection: psum = x @ [W | W@f_0 | ... | W@f_{nfold-1}] per window.
    Inputs: xTf fp16 [128, kc*W*128], Wm fp16 [kc*128, d_out],
    fv fp32 [128, nfold*d_out] (fold vectors, replicated rows).
    Output hT [128, W*(d_out+nfold)] fp16: per-window [h | dots...]."""
    DS = d_out + nfold
    nc = bacc.Bacc(num_devices=NCORES)
    xTf = nc.dram_tensor("xTf", [128, kc * W * 128], F16, kind="ExternalInput").ap()
    Wm = nc.dram_tensor("Wm", [kc * 128, d_out], F16, kind="ExternalInput").ap()
    fv = nc.dram_tensor("fv", [128, nfold * d_out], F32, kind="ExternalInput").ap()
    hT = nc.dram_tensor("hT", [128, W * DS], F16, kind="ExternalOutput").ap()

    nch = (W + PCH - 1) // PCH
    with tile.TileContext(nc) as tc:
        with (
            tc.tile_pool(name="const", bufs=1) as cpool,
            tc.tile_pool(name="ps", bufs=5, space="PSUM") as pspool,
            tc.tile_pool(name="sc", bufs=4) as scpool,
        ):
            fvb = cpool.tile([128, nfold * d_out], F32)
            nc.sync.dma_start(out=fvb[:], in_=fv[:])
            xsb = cpool.tile([128, kc * W * 128], F16)
            obig = cpool.tile([128, W * DS], F16)

            wsb = []
            for k in range(kc):
                wk = cpool.tile([128, DS], F16, tag=f"w{k}")
                nc.sync.dma_start(
                    out=wk[:, 0:d_out], in_=Wm[128 * k:128 * (k + 1), :]
                )
                for f in range(nfold):
                    scr = scpool.tile([128, d_out], F32, tag="wf")
                    nc.vector.tensor_tensor(
                        out=scr[:], in0=wk[:, 0:d_out],
                        in1=fvb[:, f * d_out:(f + 1) * d_out],
                        op=mybir.AluOpType.mult,
                    )
                    wfc = scpool.tile([128, 1], F32, tag="wfc")
                    nc.vector.reduce_sum(
                        out=wfc[:], in_=scr[:], axis=mybir.AxisListType.X
                    )
                    nc.vector.tensor_copy(
                        out=wk[:, d_out + f:d_out + f + 1], in_=wfc[:]
                    )
                wsb.append(wk)

            # chunked loads of xTf so matmuls can start early
            for ch in range(nch):
                w0, w1 = ch * PCH, min((ch + 1) * PCH, W)
                for k in range(kc):
                    nc.sync.dma_start(
                        out=xsb[:, (k * W + w0) * 128:(k * W + w1) * 128],
                        in_=xTf[:, (k * W + w0) * 128:(k * W + w1) * 128],
                    )
            for ch in range(nch):
                w0, w1 = ch * PCH, min((ch + 1) * PCH, W)
                for w in range(w0, w1):
                    ps = pspool.tile([128, DS + (DS % 2 == 0)], F32, space="PSUM")
                    for k in range(kc):
                        nc.tensor.matmul(
                            out=ps[:, 0:DS],
                            lhsT=xsb[:, (k * W + w) * 128:(k * W + w + 1) * 128],
                            rhs=wsb[k][:],
                            start=(k == 0), stop=(k == kc - 1),
                        )
                    nc.vector.tensor_copy(
                        out=obig[:, w * DS:(w + 1) * DS], in_=ps[:, 0:DS]
                    )
                nc.sync.dma_start(
                    out=hT[:, w0 * DS:w1 * DS], in_=obig[:, w0 * DS:w1 * DS]
                )
    nc.compile()
    return nc


def _build_agg(d, mA, mB, relu, uv):
    """Aggregation over one GAT layer.  Output hoT [128, W*d] fp16; with
    uv=True also uo/vo [128, W] f32 (per-node link dots, un-biased)."""
    SA = [0]
    SB = [0]
    for w in range(W):
        SA.append(SA[-1] + mA[w])
        SB.append(SB[-1] + mB[w])
    TA = -(-SA[-1] // 128)
    TB = -(-SB[-1] // 128)
    wrefs = []                       # per window: list of (region, tile)
    for w in range(W):
        rw = []
        if mA[w]:
            rw += [(0, t) for t in range(SA[w] // 128,
                                         (SA[w] + mA[w] - 1) // 128 + 1)]
        if mB[w]:
            rw += [(1, t) for t in range(SB[w] // 128,
                                         (SB[w] + mB[w] - 1) // 128 + 1)]
        wrefs.append(rw)
    NT = sum(len(rw) for rw in wrefs)
    nAc = (TA + CH - 1) // CH
    nBc = (TB + CH - 1) // CH
    K = 3 if uv else 1        # psd columns per window: [den (, u, v)]
    assert K * W <= 512
    w_last = max(w for w in range(W) if wrefs[w])

    nc = bacc.Bacc(num_devices=NCORES, num_swdge_queues=NQ)
    tableA = nc.dram_tensor("tableA", [RH, d], F16, kind="ExternalInput").ap()
    tableB = nc.dram_tensor("tableB", [RH, d], F16, kind="ExternalInput").ap()
    selfT = nc.dram_tensor("selfT", [128, W * d], F16, kind="ExternalInput").ap()
    idxA = nc.dram_tensor("idxA", [128, TA * 8], I16, kind="ExternalInput").ap()
    idxB = nc.dram_tensor("idxB", [128, TB * 8], I16, kind="ExternalInput").ap()
    dstf = nc.dram_tensor("dstf", [128, NT], F16, kind="ExternalInput").ap()
    esx = nc.dram_tensor("esx", [128, NT], F32, kind="ExternalInput").ap()
    edx = nc.dram_tensor("edx", [128, NT], F32, kind="ExternalInput").ap()
    esself = nc.dram_tensor("esself", [128, W], F32, kind="ExternalInput").ap()
    edself = nc.dram_tensor("edself", [128, W], F32, kind="ExternalInput").ap()
    iota = nc.dram_tensor("iota", [128, 128], F16, kind="ExternalInput").ap()
    iotac = nc.dram_tensor("iotac", [128, 1], F16, kind="ExternalInput").ap()
    brr = nc.dram_tensor("brr", [128, W * d], F16, kind="ExternalInput").ap()
    ho = nc.dram_tensor("ho", [128, W * d], F16, kind="ExternalOutput").ap()
    if uv:
        uph = nc.dram_tensor("uph", [128, NT * 3], F16, kind="ExternalInput").ap()
        phs = nc.dram_tensor("phs", [128, 2 * W], F16, kind="ExternalInput").ap()
        uo = nc.dram_tensor("uo", [128, W], F32, kind="ExternalOutput").ap()
        vo = nc.dram_tensor("vo", [128, W], F32, kind="ExternalOutput").ap()

    with tile.TileContext(nc) as tc:
        with (
            tc.tile_pool(name="const", bufs=1) as cpool,
            tc.tile_pool(name="ga", bufs=GBUFS) as gapool,
            tc.tile_pool(name="gb", bufs=GBUFS) as gbpool,
            tc.tile_pool(name="sp", bufs=12) as sppool,
            tc.tile_pool(name="ps", bufs=5, space="PSUM") as pspool,
            tc.tile_pool(name="psd", bufs=1, space="PSUM") as psdpool,
        ):
            idxAs = cpool.tile([128, TA * 8], I16)
            nc.sync.dma_start(out=idxAs[:], in_=idxA[:])
            idxBs = cpool.tile([128, TB * 8], I16)
            nc.sync.dma_start(out=idxBs[:], in_=idxB[:])
            dsts = cpool.tile([128, NT], F16)
            nc.sync.dma_start(out=dsts[:], in_=dstf[:])
            esxs = cpool.tile([128, NT], F32)
            nc.sync.dma_start(out=esxs[:], in_=esx[:])
            edxs = cpool.tile([128, NT], F32)
            nc.sync.dma_start(out=edxs[:], in_=edx[:])
            esss = cpool.tile([128, W], F32)
            nc.sync.dma_start(out=esss[:], in_=esself[:])
            edss = cpool.tile([128, W], F32)
            nc.sync.dma_start(out=edss[:], in_=edself[:])
            iosb = cpool.tile([128, 128], F16)
            nc.sync.dma_start(out=iosb[:], in_=iota[:])
            iocs = cpool.tile([128, 1], F16)
            nc.sync.dma_start(out=iocs[:], in_=iotac[:])
            brs = cpool.tile([128, W * d], F16)
            nc.sync.dma_start(out=brs[:], in_=brr[:])
            selfs = cpool.tile([128, W * d], F16)
            nc.sync.dma_start(out=selfs[:], in_=selfT[:])
            if uv:
                uphs = cpool.tile([128, NT * 3], F16)
                nc.sync.dma_start(out=uphs[:], in_=uph[:])
                phss = cpool.tile([128, 2 * W], F16)
                nc.sync.dma_start(out=phss[:], in_=phs[:])
            else:
                ones = cpool.tile([128, 1], F16)
                nc.vector.memset(ones[:], 1.0)
            obraw = cpool.tile([128, W * d], F16)
            obig = cpool.tile([128, W * d], F16)

            def softmax_weights(es_t, ed_t, cols, tagp):
                lg = cpool.tile([128, cols], F32, tag=f"lg{tagp}")
                nc.vector.tensor_tensor(
                    out=lg[:], in0=es_t[:], in1=ed_t[:], op=mybir.AluOpType.add
                )
                lg2 = cpool.tile([128, cols], F32, tag=f"lg2{tagp}")
                nc.vector.tensor_scalar_mul(out=lg2[:], in0=lg[:], scalar1=0.2)
                nc.vector.tensor_tensor(
                    out=lg[:], in0=lg[:], in1=lg2[:], op=mybir.AluOpType.max
                )
                p16 = cpool.tile([128, cols], F16, tag=f"p{tagp}")
                nc.scalar.activation(
                    out=p16[:], in_=lg[:], func=mybir.ActivationFunctionType.Exp
                )
                return p16

            p_all = softmax_weights(esxs, edxs, NT, "e")
            p_self = softmax_weights(esss, edss, W, "s")

            psd = psdpool.tile([128, 512], F32, space="PSUM")

            gbufA, gbufB = [], []
            emit = [0, 0]
            qctr = [0]

            def emit_chunk(region):
                k = emit[region]
                tot, pool, idxs, tab, buf = (
                    (TA, gapool, idxAs, tableA, gbufA) if region == 0
                    else (TB, gbpool, idxBs, tableB, gbufB)
                )
                t0 = k * CH
                ntiles = min(CH, tot - t0)
                gt = pool.tile([128, CH, d], F16, tag=f"g{region}")
                nc.gpsimd.dma_gather(
                    out_ap=gt[:, 0:ntiles, :], in_ap=tab[:],
                    idxs_ap=idxs[:, t0 * 8:(t0 + ntiles) * 8],
                    num_idxs=ntiles * 128, num_idxs_reg=ntiles * 128,
                    elem_size=d, queue_num=qctr[0] % NQ,
                )
                qctr[0] += 1
                buf.append(gt)
                emit[region] = k + 1

            def ensure(wtarget):
                needA = min((-(-SA[wtarget + 1] // 128) + CH - 1) // CH, nAc)
                needB = min((-(-SB[wtarget + 1] // 128) + CH - 1) // CH, nBc)
                while emit[0] < needA or emit[1] < needB:
                    if emit[0] < needA:
                        emit_chunk(0)
                    if emit[1] < needB:
                        emit_chunk(1)

            first_psd = [True]
            gcol = 0
            for w in range(W):
                ensure(min(w + LOOKAHEAD, W - 1))
                nref = len(wrefs[w])
                ps = pspool.tile([128, d + 1], F32, space="PSUM")
                sd = sppool.tile([128, 128], F16, tag="sd")
                nc.vector.scalar_tensor_tensor(
                    out=sd[:], in0=iosb[:], scalar=iocs[:, :1],
                    in1=p_self[:, w:w + 1].to_broadcast([128, 128]),
                    op0=mybir.AluOpType.is_equal, op1=mybir.AluOpType.mult,
                )
                nc.tensor.matmul(
                    out=ps[:, 0:d], lhsT=sd[:],
                    rhs=selfs[:, w * d:(w + 1) * d],
                    start=True, stop=(nref == 0),
                )
                for j, (rr, t) in enumerate(wrefs[w]):
                    rhs = (gbufA if rr == 0 else gbufB)[t // CH][:, t % CH, :]
                    sp = sppool.tile([128, 128], F16, tag="sp")
                    nc.vector.scalar_tensor_tensor(
                        out=sp[:], in0=iosb[:], scalar=dsts[:, gcol:gcol + 1],
                        in1=p_all[:, gcol:gcol + 1].to_broadcast([128, 128]),
                        op0=mybir.AluOpType.is_equal, op1=mybir.AluOpType.mult,
                    )
                    last = j == nref - 1
                    nc.tensor.matmul(
                        out=ps[:, 0:d], lhsT=sp, rhs=rhs, start=False, stop=last,
                    )
                    drhs = uphs[:, gcol * 3:gcol * 3 + 3] if uv else ones[:]
                    nc.tensor.matmul(
                        out=psd[:, w * K:w * K + K], lhsT=sp, rhs=drhs,
                        start=first_psd[0],
                        stop=(w == w_last and last),
                    )
                    first_psd[0] = False
                    gcol += 1
                nc.vector.tensor_copy(
                    out=obraw[:, w * d:(w + 1) * d], in_=ps[:, 0:d]
                )

            # ---- batched epilogue
            psd3 = psd[:, 0:K * W].rearrange("p (a b) -> p a b", a=W)
            den = cpool.tile([128, W], F32)
            nc.vector.tensor_tensor(
                out=den[:], in0=psd3[:, :, 0], in1=p_self[:],
                op=mybir.AluOpType.add,
            )
            rec = cpool.tile([128, W], F32)
            nc.vector.reciprocal(rec[:], den[:])
            for w in range(W):
                nc.vector.tensor_scalar_mul(
                    out=obig[:, w * d:(w + 1) * d],
                    in0=obraw[:, w * d:(w + 1) * d], scalar1=rec[:, w:w + 1],
                )
            nc.vector.tensor_tensor(
                out=obig[:], in0=obig[:], in1=brs[:], op=mybir.AluOpType.add
            )
            if relu:
                nc.vector.tensor_scalar_max(out=obig[:], in0=obig[:], scalar1=0.0)
            nc.sync.dma_start(out=ho[:], in_=obig[:])
            if uv:
                st = cpool.tile([128, W], F32)
                uos = cpool.tile([128, W], F32)
                vos = cpool.tile([128, W], F32)
                nc.vector.tensor_tensor(
                    out=st[:], in0=p_self[:], in1=phss[:, 0:W],
                    op=mybir.AluOpType.mult,
                )
                nc.vector.tensor_tensor(
                    out=uos[:], in0=psd3[:, :, 1], in1=st[:],
                    op=mybir.AluOpType.add,
                )
                nc.vector.tensor_tensor(
                    out=uos[:], in0=uos[:], in1=rec[:], op=mybir.AluOpType.mult
                )
                nc.vector.tensor_tensor(
                    out=st[:], in0=p_self[:], in1=phss[:, W:2 * W],
                    op=mybir.AluOpType.mult,
                )
                nc.vector.tensor_tensor(
                    out=vos[:], in0=psd3[:, :, 2], in1=st[:],
                    op=mybir.AluOpType.add,
                )
                nc.vector.tensor_tensor(
                    out=vos[:], in0=vos[:], in1=rec[:], op=mybir.AluOpType.mult
                )
                nc.sync.dma_start(out=uo[:], in_=uos[:])
                nc.sync.dma_start(out=vo[:], in_=vos[:])
    nc.compile()
    return nc


def _build_link(PT, d):
    """Link tail: z = sigmoid(um + vm + (bl + b2@wl0 + b2@wl1))."""
    nc = bacc.Bacc(num_devices=NCORES)
    um = nc.dram_tensor("um", [128, PT], F32, kind="ExternalInput").ap()
    vm = nc.dram_tensor("vm", [128, PT], F32, kind="ExternalInput").ap()
    b2r = nc.dram_tensor("b2r", [128, d], F32, kind="ExternalInput").ap()
    wl0 = nc.dram_tensor("wl0", [128, d], F32, kind="ExternalInput").ap()
    wl1 = nc.dram_tensor("wl1", [128, d], F32, kind="ExternalInput").ap()
    blr = nc.dram_tensor("blr", [128, 1], F32, kind="ExternalInput").ap()
    z = nc.dram_tensor("z", [128, PT], F32, kind="ExternalOutput").ap()
    with tile.TileContext(nc) as tc:
        with tc.tile_pool(name="c", bufs=1) as cpool:
            ums = cpool.tile([128, PT], F32)
            nc.sync.dma_start(out=ums[:], in_=um[:])
            vms = cpool.tile([128, PT], F32)
            nc.sync.dma_start(out=vms[:], in_=vm[:])
            b2s = cpool.tile([128, d], F32)
            nc.sync.dma_start(out=b2s[:], in_=b2r[:])
            w0s = cpool.tile([128, d], F32)
            nc.sync.dma_start(out=w0s[:], in_=wl0[:])
            w1s = cpool.tile([128, d], F32)
            nc.sync.dma_start(out=w1s[:], in_=wl1[:])
            bls = cpool.tile([128, 1], F32)
            nc.sync.dma_start(out=bls[:], in_=blr[:])
            scr = cpool.tile([128, d], F32)
            nc.vector.tensor_tensor(
                out=scr[:], in0=b2s[:], in1=w0s[:], op=mybir.AluOpType.mult
            )
            s0 = cpool.tile([128, 1], F32)
            nc.vector.reduce_sum(out=s0[:], in_=scr[:], axis=mybir.AxisListType.X)
            nc.vector.tensor_tensor(
                out=scr[:], in0=b2s[:], in1=w1s[:], op=mybir.AluOpType.mult
            )
            s1 = cpool.tile([128, 1], F32)
            nc.vector.reduce_sum(out=s1[:], in_=scr[:], axis=mybir.AxisListType.X)
            bb = cpool.tile([128, 1], F32)
            nc.vector.tensor_tensor(
                out=bb[:], in0=s0[:], in1=s1[:], op=mybir.AluOpType.add
            )
            nc.vector.tensor_tensor(
                out=bb[:], in0=bb[:], in1=bls[:], op=mybir.AluOpType.add
            )
            ssb = cpool.tile([128, PT], F32)
            nc.vector.tensor_tensor(
                out=ssb[:], in0=ums[:], in1=vms[:], op=mybir.AluOpType.add
            )
            zsb = cpool.tile([128, PT], F32)
            nc.scalar.activation(
                out=zsb[:], in_=ssb[:],
                func=mybir.ActivationFunctionType.Sigmoid, bias=bb[:, :1],
            )
            nc.sync.dma_start(out=z[:], in_=zsb[:])
    nc.compile()
    return nc


def _run(name, nc, in_maps, trace=True):
    last = None
    for attempt in range(3):
        try:
            res = run_bass_kernel_spmd(
                nc, in_maps, core_ids=list(range(NCORES)), trace=trace
            )
            LAST_EXEC_NS[name] = res.exec_time_ns
            return res.results
        except Exception as e:  # wedged-device retry (clears on re-attempt)
            last = e
            time.sleep(5)
    raise last


def _rep(v, n=128):
    return np.ascontiguousarray(np.broadcast_to(np.asarray(v, np.float32), (n, len(v))))


def _shard_xT(xfull):
    """[node-major, d_in] -> per-core xTf [128, kc*W*128] f16."""
    d_in = xfull.shape[1]
    kc = d_in // 128
    out = np.zeros((NCORES, 128, kc * W * 128), np.float16)
    for c in range(NCORES):
        xs = np.zeros((NSP, d_in), np.float16)
        xs[:NS] = xfull[c * NS:(c + 1) * NS]
        xt = xs.T.reshape(kc, 128, W, 128)       # [k, f, w, node]
        out[c] = xt.reshape(128, -1) if kc == 1 else np.ascontiguousarray(
            xt.transpose(1, 0, 2, 3)).reshape(128, -1)
    return out


def _split_proj(hT_list, d, nfold):
    """Per-core hT [128, W*(d+nfold)] -> (node-major h [NPAD, d] f16, per-core
    selfT [128, W*d] f16, and nfold node-major f32 dot vectors [N])."""
    DS = d + nfold
    full = np.zeros((NPAD, d), np.float16)
    selfs = []
    dots = [np.zeros(N, np.float32) for _ in range(nfold)]
    for c in range(NCORES):
        v = hT_list[c].reshape(128, W, DS)
        h = np.ascontiguousarray(v[:, :, 0:d])
        selfs.append(h.reshape(128, W * d))
        hn = h.transpose(1, 0, 2).reshape(NSP, d)
        full[c * NS:(c + 1) * NS] = hn[:NS]
        for f in range(nfold):
            dots[f][c * NS:(c + 1) * NS] = (
                v[:, :, d + f].astype(np.float32).T.ravel()[:NS]
            )
    return full, selfs, dots


def _nodevec(per_core):
    """Per-core [128, W] f32 outputs -> node-major [N] f32."""
    out = np.zeros(N, np.float32)
    for c in range(NCORES):
        out[c * NS:(c + 1) * NS] = per_core[c].T.ravel()[:NS]
    return out


# ------------------------------------------------------------------- kernel
def kernel(features, edge_index, mask, W1, a_src1, a_dst1, b1, W2, a_src2,
           a_dst2, b2, Wl, bl):
    features = np.asarray(features, np.float32)
    edge_index = np.asarray(edge_index, np.int32)
    mask = np.asarray(mask, np.int32)
    W1, W2, Wl = (np.asarray(a, np.float32) for a in (W1, W2, Wl))
    a_src1, a_dst1, b1 = (np.asarray(a, np.float32) for a in (a_src1, a_dst1, b1))
    a_src2, a_dst2, b2 = (np.asarray(a, np.float32) for a in (a_src2, a_dst2, b2))
    bl = np.asarray(bl, np.float32)

    g = _prep_graph(edge_index)
    iota = np.ascontiguousarray(
        np.broadcast_to(np.arange(128, dtype=np.float16), (128, 128))
    )
    iotac = np.arange(128, dtype=np.float16).reshape(128, 1)

    P = mask.shape[0]
    pc = P // NCORES
    PT = (pc + 127) // 128

    key = (g["mA"], g["mB"], PT)
    if key not in _PROG_CACHE:
        _PROG_CACHE[key] = dict(
            p1=_build_proj(1, H, 2),
            a1=_build_agg(H, g["mA"], g["mB"], relu=True, uv=False),
            p2=_build_proj(2, F_IN, 4),
            a2=_build_agg(F_IN, g["mA"], g["mB"], relu=False, uv=True),
            lk=_build_link(PT, F_IN),
        )
    progs = _PROG_CACHE[key]

    # ---- L1: H1 = X @ W1 (sharded), es1/ed1
    xT1 = _shard_xT(features)
    W1h = W1.astype(np.float16)
    fv1 = np.concatenate([_rep(a_src1), _rep(a_dst1)], axis=1)
    r1 = _run("p1", progs["p1"], [
        dict(xTf=xT1[c], Wm=W1h, fv=fv1) for c in range(NCORES)
    ])
    h1full, self1, (es1, ed1) = _split_proj(
        [r1[c]["hT"] for c in range(NCORES)], H, 2)

    # ---- L2: aggregate layer 1 -> h1r = relu(agg + b1)
    b1rep = np.ascontiguousarray(
        np.broadcast_to(b1.astype(np.float16), (128, W, H)).reshape(128, W * H))
    ins2 = []
    for c in range(NCORES):
        esx, edx, ess, eds = _expand(es1, ed1, g, c)
        ins2.append(dict(tableA=h1full[:RH], tableB=h1full[RH:],
                         selfT=self1[c], idxA=g["idxA"][c], idxB=g["idxB"][c],
                         dstf=g["dstf"][c], esx=esx, edx=edx,
                         esself=ess, edself=eds, iota=iota, iotac=iotac,
                         brr=b1rep))
    r2 = _run("a1", progs["a1"], ins2)
    h1rT = [r2[c]["ho"] for c in range(NCORES)]

    # ---- L3: H2 = h1r @ W2 with folds [a_s | a_d | wl0 | wl1]
    h1rfull = np.zeros((NPAD, H), np.float16)
    for c in range(NCORES):
        resh = h1rT[c].reshape(128, W, H).transpose(1, 0, 2).reshape(NSP, H)
        h1rfull[c * NS:(c + 1) * NS] = resh[:NS]
    xT2 = _shard_xT(h1rfull)
    W2h = W2.astype(np.float16)
    fv2 = np.concatenate([_rep(a_src2), _rep(a_dst2),
                          _rep(Wl[:F_IN, 0]), _rep(Wl[F_IN:, 0])], axis=1)
    r3 = _run("p2", progs["p2"], [
        dict(xTf=xT2[c], Wm=W2h, fv=fv2) for c in range(NCORES)
    ])
    h2full, self2, (es2, ed2, ph0, ph1) = _split_proj(
        [r3[c]["hT"] for c in range(NCORES)], F_IN, 4)

    # ---- L4: aggregate layer 2 -> h2 = agg + b2, psd carries [den, u, v]
    b2rep = np.ascontiguousarray(
        np.broadcast_to(b2.astype(np.float16), (128, W, F_IN)).reshape(
            128, W * F_IN))
    ins4 = []
    for c in range(NCORES):
        esx, edx, ess, eds = _expand(es2, ed2, g, c)
        pad = g["esrc"][c] < 0
        srcs = np.where(pad, 0, g["esrc"][c])
        uphc = np.zeros((128, g["NT"], 3), np.float16)
        uphc[:, :, 0] = 1.0
        uphc[:, :, 1] = ph0[srcs]
        uphc[:, :, 2] = ph1[srcs]
        nid = np.arange(NSP)
        nglob = np.minimum(c * NS + nid, N - 1)
        phsc = np.zeros((128, 2 * W), np.float16)
        phsc[:, 0:W] = np.where(nid < NS, ph0[nglob], 0.0).reshape(W, 128).T
        phsc[:, W:] = np.where(nid < NS, ph1[nglob], 0.0).reshape(W, 128).T
        ins4.append(dict(tableA=h2full[:RH], tableB=h2full[RH:],
                         selfT=self2[c], idxA=g["idxA"][c], idxB=g["idxB"][c],
                         dstf=g["dstf"][c], esx=esx, edx=edx,
                         esself=ess, edself=eds, iota=iota, iotac=iotac,
                         brr=b2rep,
                         uph=np.ascontiguousarray(uphc.reshape(128, -1)),
                         phs=phsc))
    r4 = _run("a2", progs["a2"], ins4)
    ufull = _nodevec([r4[c]["uo"] for c in range(NCORES)])
    vfull = _nodevec([r4[c]["vo"] for c in range(NCORES)])

    # ---- L5: link predictor tail on host-gathered u/v scalars
    mT = mask.T
    um = np.zeros((NCORES, 128, PT), np.float32)
    vm = np.zeros((NCORES, 128, PT), np.float32)
    for c in range(NCORES):
        s = np.arange(pc)
        um[c, s % 128, s // 128] = ufull[mT[0][c * pc:(c + 1) * pc]]
        vm[c, s % 128, s // 128] = vfull[mT[1][c * pc:(c + 1) * pc]]
    blr = np.full((128, 1), float(bl[0]), np.float32)
    r5 = _run("lk", progs["lk"], [
        dict(um=um[c], vm=vm[c], b2r=_rep(b2), wl0=_rep(Wl[:F_IN, 0]),
             wl1=_rep(Wl[F_IN:, 0]), blr=blr)
        for c in range(NCORES)
    ])
    out = np.zeros((P, 1), np.float32)
    for c in range(NCORES):
        s = np.arange(pc)
        out[c * pc:(c + 1) * pc, 0] = r5[c]["z"][s % 128, s // 128]
    tot = sum(v for v in LAST_EXEC_NS.values() if v)
    print(f"kernel launches ns: {LAST_EXEC_NS} total {tot}")
    return out
